# revision 1
# baseline (speedup 1.0000x reference)
"""Multi-head attention (B=4, S=2048, E=1024, H=16, D=64) on 8 trn2 cores.

Sharding: 2D (batch x head-group). Core c handles batch b = c//2 and head
group g = c%2 (8 heads = 512 feature dims). Each core computes a full
[S, E] partial of the output projection for its batch; the host sums the
two group partials per batch and adds the bias.

Per-core device kernel (all fp16/bf16 matmuls, fp32 PSUM accumulation):
  qT = (Wq_loc @ X_q^T)      [512, 2048]  (stored transposed, fp16)
  kT = (Wk_loc @ X_k^T)      [512, 2048]
  v  = X_v @ Wv_loc^T        [2048, 512]  (natural layout + ones column, bf16)
  per head h, per qq-tile (1024), per kk-chunk (128):
    scores^T chunk = kT_h_chunk.T @ qT_h   -> PSUM [128, 1024] f32
    P^T = exp(scores^T)  (ScalarE, no max subtraction: max score ~76,
                          exp fits fp32/bf16 range)   -> SBUF bf16
    U  += v_aug_chunk.T @ P^T  -> PSUM [65, 1024] f32  (row 64 = softmax denom)
  attnout^T = U[0:64] * broadcast(1/U[64])  -> SBUF fp16
  y = attnout^T.T @ Wo_loc^T  -> [2048, 1024] f32 partial
"""

from contextlib import ExitStack

import numpy as np

S = 2048
E = 1024
F = 512          # local feature dims (8 heads x 64)
HL = 8           # heads per core
D = 64
B = 4
H = 16
NCORES = 8

_CACHE = {}


def build_nc(reps: int = 1):
    import concourse.tile as tile
    from concourse import bacc, mybir

    F16 = mybir.dt.float16
    BF16 = mybir.dt.bfloat16
    F32 = mybir.dt.float32
    EXP = mybir.ActivationFunctionType.Exp

    nc = bacc.Bacc(
        "TRN2",
        target_bir_lowering=False,
        debug=False,
        enable_asserts=False,
        num_devices=NCORES,
    )

    xq_d = nc.dram_tensor("xq", [E, S], F16, kind="ExternalInput").ap()
    xk_d = nc.dram_tensor("xk", [E, S], F16, kind="ExternalInput").ap()
    xv_d = nc.dram_tensor("xv", [E, S], F16, kind="ExternalInput").ap()
    wq_d = nc.dram_tensor("wq", [E, F], F16, kind="ExternalInput").ap()
    wk_d = nc.dram_tensor("wk", [E, F], F16, kind="ExternalInput").ap()
    wv_d = nc.dram_tensor("wv", [E, F], F16, kind="ExternalInput").ap()
    wo_d = nc.dram_tensor("wo", [F, E], F16, kind="ExternalInput").ap()
    y_d = nc.dram_tensor("y", [S, E], F32, kind="ExternalOutput").ap()

    with tile.TileContext(nc) as tc, ExitStack() as ctx:
        persist = ctx.enter_context(tc.tile_pool(name="persist", bufs=1))
        xpool = ctx.enter_context(tc.tile_pool(name="xpool", bufs=16))
        ppool = ctx.enter_context(tc.tile_pool(name="ppool", bufs=4))
        ypool = ctx.enter_context(tc.tile_pool(name="ypool", bufs=3))
        smpool = ctx.enter_context(tc.tile_pool(name="smpool", bufs=2))
        ps_s = ctx.enter_context(tc.tile_pool(name="ps_s", bufs=2, space="PSUM"))
        ps_u = ctx.enter_context(tc.tile_pool(name="ps_u", bufs=2, space="PSUM"))

        def body(iv):
            # ---------------- weight/x loads ----------------
            def load_w(dram, pfx, width):
                tiles = []
                nchunks = dram.shape[0] // 128
                for i in range(nchunks):
                    t = persist.tile([128, width], F16, tag=f"{pfx}{i}",
                                     name=f"{pfx}_sb{i}")
                    nc.sync.dma_start(t[:], dram[i * 128:(i + 1) * 128, :])
                    tiles.append(t)
                return tiles

            def load_x(dram, pfx):
                tiles = []
                for eci in range(8):
                    t = xpool.tile([128, S], F16, tag="x", name=f"x{pfx}{eci}")
                    nc.sync.dma_start(t[:], dram[eci * 128:(eci + 1) * 128, :])
                    tiles.append(t)
                return tiles

            # v first (attention depends on all of v); interleave w/x DMAs
            # so the first v-proj matmul starts after ~0.7 MB instead of 5 MB
            wv_sb, xv_sb = [], []
            for eci in range(8):
                t = persist.tile([128, F], F16, tag=f"wv{eci}",
                                 name=f"wv_sb{eci}")
                nc.sync.dma_start(t[:], wv_d[eci * 128:(eci + 1) * 128, :])
                wv_sb.append(t)
                t = xpool.tile([128, S], F16, tag="x", name=f"xv{eci}")
                nc.sync.dma_start(t[:], xv_d[eci * 128:(eci + 1) * 128, :])
                xv_sb.append(t)

            # v with ones column: v_sb[p, tc, h, d] = v[tc*128+p, h*64+d],
            # d=64 column stays 1.0 (softmax denominator trick)
            v_sb = persist.tile([128, 16, HL, D + 1], BF16, tag="v_sb",
                                name="v_sb")
            nc.vector.memset(v_sb[:], 1.0)

            # ---------------- V projection (natural layout) ----------------
            for tci in range(16):
                vp = ps_u.tile([128, F], F32, tag="u", name=f"v_ps{tci}")
                for eci in range(8):
                    nc.tensor.matmul(
                        vp[:],
                        lhsT=xv_sb[eci][:, tci * 128:(tci + 1) * 128],
                        rhs=wv_sb[eci][:],
                        start=(eci == 0),
                        stop=(eci == 7),
                    )
                for h in range(HL):
                    nc.vector.tensor_copy(v_sb[:, tci, h, 0:D],
                                          vp[:, h * D:(h + 1) * D])

            # ---------------- Q/K projection chunks (transposed layout) ----
            wq_sb = load_w(wq_d, "wq", F)
            xq_sb = load_x(xq_d, "q")
            wk_sb = load_w(wk_d, "wk", F)
            xk_sb = load_x(xk_d, "k")
            wo_sb = load_w(wo_d, "wo", E)

            qT_sb = [persist.tile([128, S], F16, tag=f"qT{i}", name=f"qT_sb{i}")
                     for i in range(4)]
            kT_sb = [persist.tile([128, S], F16, tag=f"kT{i}", name=f"kT_sb{i}")
                     for i in range(4)]

            def proj_oc(x_sb, w_sb, ot, oci, pfx):
                for half in range(2):
                    pp = ps_s.tile([128, 1024], F32, tag="s",
                                   name=f"{pfx}p{oci}_{half}")
                    for eci in range(8):
                        for nb in range(2):
                            col = half * 1024 + nb * 512
                            nc.tensor.matmul(
                                pp[:, nb * 512:(nb + 1) * 512],
                                lhsT=w_sb[eci][:, oci * 128:(oci + 1) * 128],
                                rhs=x_sb[eci][:, col:col + 512],
                                start=(eci == 0),
                                stop=(eci == 7),
                            )
                    nc.vector.tensor_copy(
                        ot[:, half * 1024:(half + 1) * 1024], pp[:])

            # attnout^T storage
            aT_sb = [persist.tile([128, S], F16, tag=f"aT{i}", name=f"aT_sb{i}")
                     for i in range(4)]

            # ---------------- attention for one head ----------------
            # Both qq halves (qt=0,1) processed jointly: two interleaved
            # exp streams keep ACT saturated while each stream's scores
            # PSUM tile is effectively single-buffered (4+4 banks total).
            def attn_head(h):
                ch, hh = h // 2, h % 2
                p0, p1 = hh * 64, hh * 64 + 64
                U = [ps_u.tile([65, 1024], F32, tag="u", name=f"U{h}_{qt}")
                     for qt in range(2)]
                prev = [None, None]

                def av(qt, kk, pt):
                    for nb in range(2):
                        nc.tensor.matmul(
                            U[qt][:, nb * 512:(nb + 1) * 512],
                            lhsT=v_sb[:, kk, h, :],
                            rhs=pt[:, nb * 512:(nb + 1) * 512],
                            start=(kk == 0),
                            stop=(kk == 15),
                        )

                for kk in range(16):
                    sc = [None, None]
                    for qt in range(2):
                        s = ps_s.tile([128, 1024], F32, tag="s",
                                      name=f"sc{h}_{qt}_{kk}")
                        for nb in range(2):
                            qcol = qt * 1024 + nb * 512
                            nc.tensor.matmul(
                                s[:, nb * 512:(nb + 1) * 512],
                                lhsT=kT_sb[ch][p0:p1, kk * 128:(kk + 1) * 128],
                                rhs=qT_sb[ch][p0:p1, qcol:qcol + 512],
                                start=True,
                                stop=True,
                            )
                        sc[qt] = s
                        # AV of previous chunk emitted between the two score
                        # streams so the PE always has ready work
                        if prev[qt] is not None:
                            av(qt, kk - 1, prev[qt])
                    for qt in range(2):
                        pt = ppool.tile([128, 1024], BF16, tag="p",
                                        name=f"p{h}_{qt}_{kk}")
                        nc.scalar.activation(pt[:], sc[qt][:], EXP)
                        prev[qt] = pt
                for qt in range(2):
                    av(qt, 15, prev[qt])

                # normalize: aT = U[0:64] / U[64]
                for qt in range(2):
                    rcp = smpool.tile([1, 1024], F32, tag="rcp",
                                      name=f"rcp{h}_{qt}")
                    nc.vector.reciprocal(rcp[:], U[qt][64:65, :])
                    bc = smpool.tile([64, 1024], F32, tag="bc",
                                     name=f"bc{h}_{qt}")
                    nc.gpsimd.partition_broadcast(bc[:], rcp[:])
                    nc.vector.tensor_mul(
                        aT_sb[ch][p0:p1, qt * 1024:(qt + 1) * 1024],
                        U[qt][0:64, :], bc[:])

            # First q/k chunk upfront, later chunks interleaved at head
            # boundaries (chunk p is needed from head 2p onward).
            proj_oc(xq_sb, wq_sb, qT_sb[0], 0, "q")
            proj_oc(xk_sb, wk_sb, kT_sb[0], 0, "k")
            attn_head(0)
            proj_oc(xq_sb, wq_sb, qT_sb[1], 1, "q")
            attn_head(1)
            proj_oc(xk_sb, wk_sb, kT_sb[1], 1, "k")
            attn_head(2)
            proj_oc(xq_sb, wq_sb, qT_sb[2], 2, "q")
            attn_head(3)
            proj_oc(xk_sb, wk_sb, kT_sb[2], 2, "k")
            attn_head(4)
            proj_oc(xq_sb, wq_sb, qT_sb[3], 3, "q")
            attn_head(5)
            proj_oc(xk_sb, wk_sb, kT_sb[3], 3, "k")
            attn_head(6)
            attn_head(7)

            # ---------------- output projection ----------------
            for tci in range(16):
                yp = ps_u.tile([128, 1024], F32, tag="u", name=f"y_ps{tci}")
                for fc in range(4):
                    for nb in range(2):
                        nc.tensor.matmul(
                            yp[:, nb * 512:(nb + 1) * 512],
                            lhsT=aT_sb[fc][:, tci * 128:(tci + 1) * 128],
                            rhs=wo_sb[fc][:, nb * 512:(nb + 1) * 512],
                            start=(fc == 0),
                            stop=(fc == 3),
                        )
                ysb = ypool.tile([128, 1024], F32, tag="y", name=f"y_sb{tci}")
                # ACT is idle during the output projection; split the PSUM
                # drain copies between ACT and DVE
                if tci % 2 == 0:
                    nc.scalar.copy(ysb[:], yp[:])
                else:
                    nc.vector.tensor_copy(ysb[:], yp[:])
                nc.sync.dma_start(y_d[tci * 128:(tci + 1) * 128, :], ysb[:])

        if reps == 1:
            body(0)
        else:
            with tc.For_i(0, reps, 1) as iv:
                body(iv)

    nc.compile()
    return nc


def make_in_maps(Q, K, V, Wq, Wk, Wv, Wo):
    """Shard + lay out full inputs for the 8 cores."""
    Q = np.asarray(Q, dtype=np.float32)
    K = np.asarray(K, dtype=np.float32)
    V = np.asarray(V, dtype=np.float32)
    Wq = np.asarray(Wq, dtype=np.float32)
    Wk = np.asarray(Wk, dtype=np.float32)
    Wv = np.asarray(Wv, dtype=np.float32)
    Wo = np.asarray(Wo, dtype=np.float32)

    in_maps = []
    for c in range(NCORES):
        b, g = c // 2, c % 2
        rows = slice(g * F, (g + 1) * F)
        in_maps.append({
            "xq": np.ascontiguousarray(Q[b].T).astype(np.float16),
            "xk": np.ascontiguousarray(K[b].T).astype(np.float16),
            "xv": np.ascontiguousarray(V[b].T).astype(np.float16),
            "wq": np.ascontiguousarray(Wq[rows, :].T).astype(np.float16),
            "wk": np.ascontiguousarray(Wk[rows, :].T).astype(np.float16),
            "wv": np.ascontiguousarray(Wv[rows, :].T).astype(np.float16),
            "wo": np.ascontiguousarray(Wo[:, rows].T).astype(np.float16),
        })
    return in_maps


def combine(results, bo):
    """Sum per-core partials + bias -> full [B, S, E] output."""
    bo = np.asarray(bo, dtype=np.float32)
    y = np.zeros((B, S, E), dtype=np.float32)
    for c in range(NCORES):
        y[c // 2] += results[c]["y"]
    y += bo[None, None, :]
    return y


def kernel(Q, K, V, Wq, Wk, Wv, Wo, bo):
    from concourse.bass_utils import run_bass_kernel_spmd

    if "nc" not in _CACHE:
        _CACHE["nc"] = build_nc(reps=1)
    nc = _CACHE["nc"]
    in_maps = make_in_maps(Q, K, V, Wq, Wk, Wv, Wo)
    res = run_bass_kernel_spmd(nc, in_maps, core_ids=list(range(NCORES)))
    return combine(res.results, bo)



# revision 23
# speedup vs baseline: 1.5701x; 1.5701x over previous
"""Multi-head attention (B=4, S=2048, E=1024, H=16, D=64) on 8 trn2 cores.

Sharding: 2D (batch x head-group). Core c handles batch b = c//2 and head
group g = c%2 (8 heads = 512 feature dims). Each core computes a full
[S, E] partial of the output projection for its batch; the host sums the
two group partials per batch and adds the bias.

v3 design (ACT-exp is the pacer at ~266us/core; keep PE hidden under it):
  - scores: kk-parity PAIRS of (64,128) row-tile matmuls running
    concurrently on both halves of the PE array. Head h lives in one
    64-partition half of qT/kT chunk ch=h//2; a swapped copy (qTB/kTB,
    made by SBUF->SBUF DMA) provides the same head in the other half so
    even kk chunks use tile (hh*64,0) and odd chunks tile ((1-hh)*64,0).
  - attention is qt-sequential: slot = (h, qt, kk-pair); exp slices are
    [128,1024] f32 PSUM -> bf16 SBUF.
  - AV: full-array matmuls with the ones-column denominator trick
    (v stationary [128, 65]); single PSUM U [65,1024] per (h,qt), eagerly
    evacuated to SBUF by DVE so the next (h,qt) can start.
  - Q/K projections for chunks 1-3 are sliced into (eci)-granular filler
    units popped ~2 per attention slot, so the PE works while ACT exps.
  - O-projection: fc01 partial matmuls run as filler during heads 4-7
    (into bf16 staging tiles); fc23 + merge at the tail.
PSUM budget: scores 2x[128,1024]=4 banks, U [65,1024]=2, proj [128,1024]=2.
"""

from collections import deque
from contextlib import ExitStack

import numpy as np

S = 2048
E = 1024
F = 512          # local feature dims (8 heads x 64)
HL = 8           # heads per core
D = 64
B = 4
H = 16
NCORES = 8

_CACHE = {}

DEBUG_DUMP = False


def build_nc(reps: int = 1):
    import concourse.tile as tile
    from concourse import bacc, mybir

    F16 = mybir.dt.float16
    BF16 = mybir.dt.bfloat16
    F32 = mybir.dt.float32
    EXP = mybir.ActivationFunctionType.Exp

    nc = bacc.Bacc(
        "TRN2",
        target_bir_lowering=False,
        debug=False,
        enable_asserts=False,
        num_devices=NCORES,
    )

    xq_d = nc.dram_tensor("xq", [E, S], F16, kind="ExternalInput").ap()
    xk_d = nc.dram_tensor("xk", [E, S], F16, kind="ExternalInput").ap()
    xv_d = nc.dram_tensor("xv", [E, S], F16, kind="ExternalInput").ap()
    wq_d = nc.dram_tensor("wq", [E, F], F16, kind="ExternalInput").ap()
    wk_d = nc.dram_tensor("wk", [E, F], F16, kind="ExternalInput").ap()
    wv_d = nc.dram_tensor("wv", [E, F], F16, kind="ExternalInput").ap()
    wo_d = nc.dram_tensor("wo", [F, E], F16, kind="ExternalInput").ap()
    # y in bf16: host upcasts + sums partials in f32 (err ~0.4% << 2e-2)
    y_d = nc.dram_tensor("y", [S, E], BF16, kind="ExternalOutput").ap()
    dbg = {}
    if DEBUG_DUMP:
        for nm in ("qTA", "kTA", "qTB", "kTB", "aT"):
            dbg[nm] = nc.dram_tensor(f"dbg_{nm}", [F, S], F16,
                                     kind="ExternalOutput").ap()
        dbg["usb"] = nc.dram_tensor("dbg_usb", [65, 1024], F32,
                                    kind="ExternalOutput").ap()

    with tile.TileContext(nc) as tc, ExitStack() as ctx:
        persist = ctx.enter_context(tc.tile_pool(name="persist", bufs=1))
        xpool = ctx.enter_context(tc.tile_pool(name="xpool", bufs=16))
        ppool = ctx.enter_context(tc.tile_pool(name="ppool", bufs=4))
        ypool = ctx.enter_context(tc.tile_pool(name="ypool", bufs=2))
        smpool = ctx.enter_context(tc.tile_pool(name="smpool", bufs=1))
        ps_sc = ctx.enter_context(tc.tile_pool(name="ps_sc", bufs=2,
                                               space="PSUM"))
        ps_u = ctx.enter_context(tc.tile_pool(name="ps_u", bufs=1,
                                              space="PSUM"))
        ps_w = ctx.enter_context(tc.tile_pool(name="ps_w", bufs=1,
                                              space="PSUM"))

        def body(iv):
            # ---------------- weight loads ----------------
            def load_w(dram, pfx, width):
                tiles = []
                for i in range(dram.shape[0] // 128):
                    t = persist.tile([128, width], F16, tag=f"{pfx}{i}",
                                     name=f"{pfx}_sb{i}")
                    nc.sync.dma_start(t[:], dram[i * 128:(i + 1) * 128, :])
                    tiles.append(t)
                return tiles

            def load_x(dram, pfx):
                tiles = []
                for eci in range(8):
                    t = xpool.tile([128, S], F16, tag="x", name=f"x{pfx}{eci}")
                    nc.sync.dma_start(t[:], dram[eci * 128:(eci + 1) * 128, :])
                    tiles.append(t)
                return tiles

            # v first (attention needs v earliest); interleave wv/xv DMAs
            wv_sb, xv_sb = [], []
            for eci in range(8):
                t = xpool.tile([128, F], F16, tag="x", name=f"wv_sb{eci}")
                nc.sync.dma_start(t[:], wv_d[eci * 128:(eci + 1) * 128, :])
                wv_sb.append(t)
                t = xpool.tile([128, S], F16, tag="x", name=f"xv{eci}")
                nc.sync.dma_start(t[:], xv_d[eci * 128:(eci + 1) * 128, :])
                xv_sb.append(t)

            # v with ones column: v_sb[p, tc, h, d] = v[tc*128+p, h*64+d],
            # d=64 column stays 1.0 (softmax denominator trick)
            v_sb = persist.tile([128, 16, HL, D + 1], BF16, tag="v_sb",
                                name="v_sb")
            nc.vector.memset(v_sb[:], 1.0)

            # ---------------- V projection ----------------
            # V-proj runs pre-attention: borrow the (idle) scores pool so it
            # double-buffers; ps_w stays free for the Q/K chunk-0 pipeline
            for tci in range(16):
                vp = ps_sc.tile([128, F], F32, tag="s", name=f"v_ps{tci}")
                for eci in range(8):
                    nc.tensor.matmul(
                        vp[:],
                        lhsT=xv_sb[eci][:, tci * 128:(tci + 1) * 128],
                        rhs=wv_sb[eci][:],
                        start=(eci == 0),
                        stop=(eci == 7),
                    )
                # one strided DVE copy moves all 8 heads into v_sb layout
                # (dst free dims (8,64) walk h-major, matching vp columns)
                nc.vector.tensor_copy(v_sb[:, tci, :, 0:D], vp[:])

            wq_sb = load_w(wq_d, "wq", F)
            xq_sb = load_x(xq_d, "q")
            wk_sb = load_w(wk_d, "wk", F)
            xk_sb = load_x(xk_d, "k")
            wo_sb = load_w(wo_d, "wo", E)

            # qT/kT chunk tiles: A = natural (head 2ch in partitions 0-63,
            # head 2ch+1 in 64-127), B = halves swapped (via SBUF-SBUF DMA)
            def qk_tiles(pfx):
                return [persist.tile([128, S], F16, tag=f"{pfx}{i}",
                                     name=f"{pfx}_sb{i}") for i in range(4)]

            qTA, kTA = qk_tiles("qTA"), qk_tiles("kTA")
            qTB, kTB = qk_tiles("qTB"), qk_tiles("kTB")

            # attnout^T storage
            aT_sb = [persist.tile([128, S], F16, tag=f"aT{i}", name=f"aT_sb{i}")
                     for i in range(4)]

            # ---------------- Q/K projection units ----------------
            # chunk (which, oci): out rows oci*128..+128 of qT/kT.
            # Emitted as filler units: per half (x-cols 0-1023 / 1024-2047):
            # 8 eci matmul units + 1 drain unit (DVE copy + 2 dup DMAs).
            def make_qk_units(w_sb, x_sb, tA, tB, oci, pfx):
                units = []
                state = {}

                def mk_mm(half, eci):
                    def emit():
                        if eci == 0:
                            state[half] = ps_w.tile(
                                [128, 1024], F32, tag="w",
                                name=f"{pfx}p{oci}_{half}")
                        pp = state[half]
                        for nb in range(2):
                            col = half * 1024 + nb * 512
                            nc.tensor.matmul(
                                pp[:, nb * 512:(nb + 1) * 512],
                                lhsT=w_sb[eci][:, oci * 128:(oci + 1) * 128],
                                rhs=x_sb[eci][:, col:col + 512],
                                start=(eci == 0),
                                stop=(eci == 7),
                            )
                    return emit

                def mk_drain(half):
                    def emit():
                        pp = state[half]
                        cols = slice(half * 1024, (half + 1) * 1024)
                        nc.vector.tensor_copy(tA[:, cols], pp[:])
                        # swapped-halves copy for the row-tile pairing
                        nc.sync.dma_start(tB[0:64, cols], tA[64:128, cols])
                        nc.sync.dma_start(tB[64:128, cols], tA[0:64, cols])
                    return emit

                for half in range(2):
                    for eci in range(8):
                        units.append(mk_mm(half, eci))
                    units.append(mk_drain(half))
                return units

            # fix tA indexing helper (tile AP slicing)
            def qk_units(which, oci):
                if which == "q":
                    return make_qk_units(wq_sb, xq_sb, qTA[oci], qTB[oci],
                                         oci, "q")
                return make_qk_units(wk_sb, xk_sb, kTA[oci], kTB[oci],
                                     oci, "k")

            filler_q = deque()

            def pop_filler(n):
                for _ in range(n):
                    if not filler_q:
                        return
                    filler_q.popleft()()

            # chunk 0 upfront (attention head 0 needs it)
            for u in qk_units("q", 0):
                u()
            for u in qk_units("k", 0):
                u()
            # chunks 1-3 queued as filler
            for oci in range(1, 4):
                filler_q.extend(qk_units("q", oci))
                filler_q.extend(qk_units("k", oci))

            # ---------------- O-projection staging ----------------
            y01_sb = []

            def make_o01_units(tci):
                units = []
                state = {}

                def mk_mm(fc):
                    def emit():
                        if fc == 0:
                            state["p"] = ps_w.tile([128, 1024], F32, tag="w",
                                                   name=f"y01_ps{tci}")
                        yp = state["p"]
                        for nb in range(2):
                            nc.tensor.matmul(
                                yp[:, nb * 512:(nb + 1) * 512],
                                lhsT=aT_sb[fc][:, tci * 128:(tci + 1) * 128],
                                rhs=wo_sb[fc][:, nb * 512:(nb + 1) * 512],
                                start=(fc == 0),
                                stop=(fc == 1),
                            )
                    return emit

                def mk_drain():
                    def emit():
                        t = xpool.tile([128, 1024], BF16, tag="x",
                                       name=f"y01_sb{tci}")
                        y01_sb.append(t)
                        nc.vector.tensor_copy(t[:], state["p"][:])
                    return emit

                units.append(mk_mm(0))
                units.append(mk_mm(1))
                units.append(mk_drain())
                return units

            # ---------------- attention ----------------
            def attn_head_qt(h, qt):
                ch, hh = h // 2, h % 2
                pA = hh * 64        # head h's half in the A copies
                pB = (1 - hh) * 64  # head h's half in the B copies
                qc0 = qt * 1024

                U = ps_u.tile([65, 1024], F32, tag="u", name=f"U{h}_{qt}")
                prev = None

                def av(kk, pt):
                    for nb in range(2):
                        nc.tensor.matmul(
                            U[:, nb * 512:(nb + 1) * 512],
                            lhsT=v_sb[:, kk, h, :],
                            rhs=pt[:, nb * 512:(nb + 1) * 512],
                            start=(kk == 0),
                            stop=(kk == 15),
                        )

                for kp in range(8):
                    kk0, kk1 = 2 * kp, 2 * kp + 1
                    s_ev = ps_sc.tile([128, 1024], F32, tag="s",
                                      name=f"sc{h}_{qt}_{kk0}")
                    s_od = ps_sc.tile([128, 1024], F32, tag="s",
                                      name=f"sc{h}_{qt}_{kk1}")
                    # scores pair: concurrent row tiles (pA vs pB halves)
                    for nb in range(2):
                        qc = qc0 + nb * 512
                        nc.tensor.matmul(
                            s_ev[:, nb * 512:(nb + 1) * 512],
                            lhsT=kTA[ch][pA:pA + 64,
                                         kk0 * 128:(kk0 + 1) * 128],
                            rhs=qTA[ch][pA:pA + 64, qc:qc + 512],
                            start=True, stop=True,
                        )
                    for nb in range(2):
                        qc = qc0 + nb * 512
                        nc.tensor.matmul(
                            s_od[:, nb * 512:(nb + 1) * 512],
                            lhsT=kTB[ch][pB:pB + 64,
                                         kk1 * 128:(kk1 + 1) * 128],
                            rhs=qTB[ch][pB:pB + 64, qc:qc + 512],
                            start=True, stop=True,
                        )
                    # exp of this pair (ACT is the pacer)
                    pt_ev = ppool.tile([128, 1024], BF16, tag="p",
                                       name=f"p{h}_{qt}_{kk0}")
                    nc.scalar.activation(pt_ev[:], s_ev[:], EXP)
                    pt_od = ppool.tile([128, 1024], BF16, tag="p",
                                       name=f"p{h}_{qt}_{kk1}")
                    nc.scalar.activation(pt_od[:], s_od[:], EXP)
                    # AV of the previous pair runs while ACT exps
                    if prev is not None:
                        av(prev[0], prev[1])
                        av(prev[2], prev[3])
                    prev = (kk0, pt_ev, kk1, pt_od)
                    pop_filler(2)
                av(prev[0], prev[1])
                av(prev[2], prev[3])

                # evacuate U then normalize from SBUF (frees PSUM quickly)
                U_sb = smpool.tile([65, 1024], F32, tag="usb",
                                   name=f"Usb{h}_{qt}")
                nc.vector.tensor_copy(U_sb[0:64, :], U[0:64, :])
                # reciprocal of the PSUM denominator row into a tile that
                # STARTS at partition 0 (partition_broadcast reads physical
                # partition 0 of its source)
                rcp = smpool.tile([1, 1024], BF16, tag="rcp",
                                  name=f"rcp{h}_{qt}")
                with nc.allow_low_precision(reason="1/denom in bf16 is fine"):
                    nc.vector.reciprocal(rcp[:], U[64:65, :])
                if DEBUG_DUMP and h == 0 and qt == 0:
                    nc.sync.dma_start(dbg["usb"][:], U_sb[:])
                bc = smpool.tile([64, 1024], BF16, tag="bc", name=f"bc{h}_{qt}")
                nc.gpsimd.partition_broadcast(bc[:], rcp[:])
                nc.vector.tensor_mul(
                    aT_sb[ch][pA:pA + 64, qc0:qc0 + 1024],
                    U_sb[0:64, :], bc[:])

            for h in range(8):
                for qt in range(2):
                    attn_head_qt(h, qt)
                if h == 3:
                    # aT chunks 0,1 complete: queue O-proj fc01 partials
                    for tci in range(16):
                        filler_q.extend(make_o01_units(tci))

            # drain any remaining filler (shouldn't be much)
            pop_filler(len(filler_q))

            if DEBUG_DUMP:
                for nm, tiles in (("qTA", qTA), ("kTA", kTA), ("qTB", qTB),
                                  ("kTB", kTB), ("aT", aT_sb)):
                    for i in range(4):
                        nc.sync.dma_start(
                            dbg[nm][i * 128:(i + 1) * 128, :], tiles[i][:])

            # ---------------- O-projection tail: fc2,3 + merge ----------
            # scores pool is idle at the tail: use it for double-buffering
            for tci in range(16):
                yp = ps_sc.tile([128, 1024], F32, tag="s", name=f"y23_ps{tci}")
                for fc in (2, 3):
                    for nb in range(2):
                        nc.tensor.matmul(
                            yp[:, nb * 512:(nb + 1) * 512],
                            lhsT=aT_sb[fc][:, tci * 128:(tci + 1) * 128],
                            rhs=wo_sb[fc][:, nb * 512:(nb + 1) * 512],
                            start=(fc == 2),
                            stop=(fc == 3),
                        )
                ysb = ypool.tile([128, 1024], BF16, tag="y", name=f"y_sb{tci}")
                # y = fc23 partial (psum) + fc01 partial (bf16 sbuf)
                nc.vector.tensor_add(ysb[:], yp[:], y01_sb[tci][:])
                nc.sync.dma_start(y_d[tci * 128:(tci + 1) * 128, :], ysb[:])

        if reps == 1:
            body(0)
        else:
            with tc.For_i(0, reps, 1) as iv:
                body(iv)

    nc.compile()
    return nc


def make_in_maps(Q, K, V, Wq, Wk, Wv, Wo):
    """Shard + lay out full inputs for the 8 cores."""
    Q = np.asarray(Q, dtype=np.float32)
    K = np.asarray(K, dtype=np.float32)
    V = np.asarray(V, dtype=np.float32)
    Wq = np.asarray(Wq, dtype=np.float32)
    Wk = np.asarray(Wk, dtype=np.float32)
    Wv = np.asarray(Wv, dtype=np.float32)
    Wo = np.asarray(Wo, dtype=np.float32)

    in_maps = []
    for c in range(NCORES):
        b, g = c // 2, c % 2
        rows = slice(g * F, (g + 1) * F)
        in_maps.append({
            "xq": np.ascontiguousarray(Q[b].T).astype(np.float16),
            "xk": np.ascontiguousarray(K[b].T).astype(np.float16),
            "xv": np.ascontiguousarray(V[b].T).astype(np.float16),
            "wq": np.ascontiguousarray(Wq[rows, :].T).astype(np.float16),
            "wk": np.ascontiguousarray(Wk[rows, :].T).astype(np.float16),
            "wv": np.ascontiguousarray(Wv[rows, :].T).astype(np.float16),
            "wo": np.ascontiguousarray(Wo[:, rows].T).astype(np.float16),
        })
    return in_maps


def combine(results, bo):
    """Sum per-core partials + bias -> full [B, S, E] output."""
    bo = np.asarray(bo, dtype=np.float32)
    y = np.zeros((B, S, E), dtype=np.float32)
    for c in range(NCORES):
        y[c // 2] += np.asarray(results[c]["y"]).astype(np.float32)
    y += bo[None, None, :]
    return y


def kernel(Q, K, V, Wq, Wk, Wv, Wo, bo):
    from concourse.bass_utils import run_bass_kernel_spmd

    if "nc" not in _CACHE:
        _CACHE["nc"] = build_nc(reps=1)
    nc = _CACHE["nc"]
    in_maps = make_in_maps(Q, K, V, Wq, Wk, Wv, Wo)
    res = run_bass_kernel_spmd(nc, in_maps, core_ids=list(range(NCORES)))
    return combine(res.results, bo)
